# revision 1
# baseline (speedup 1.0000x reference)
"""Trainium2 Bass kernel for nn_BottomLevelDecoderRNN.

2-layer GRU decoder, H=1024, over S=16 steps for E*B = 16*128 = 2048
independent sequences. Data-parallel over 8 NeuronCores: each core owns
2 conductor embeddings (256 rows).

Dataflow (per core, everything transposed as [feature, row]):
  init:    h-init: t0T = tanh(fc_init_w @ cflatT + b)  -> h1T, h2T [H, 256]
           cached = Wc @ cflatT + bih1                 (Wc = g1_wih[:, :C])
  step s:  g1T = Wp @ prevT[s] + whh1 @ h1T + cached   -> GRU1 gates -> h1T'
           g2T = wih2 @ h1T' + whh2 @ h2T              -> GRU2 gates -> h2T'
           yT  = fco_w @ h2T' + fco_b                  -> out
Matmuls run with fp16 operands (weights pre-transposed on host into PE
stationary-tile layout), fp32 PSUM accumulation. whh2 streams from HBM
per step (SBUF cannot hold all three 3072x1024 matrices + state).
"""
import numpy as np

E, B, C, H, D = 16, 128, 512, 1024, 130
S = 16
NCORES = 8
EPC = E // NCORES        # 2 embeddings per core
R = EPC * B              # 256 rows per core
KH = H // 128            # 8 h k-tiles
MG = 3 * H // 128        # 24 gate m-tiles
MRZ = 2 * H // 128       # 16 rz m-tiles
NJ = H // 128            # 8 n/h tiles
KC = C // 128            # 4 c k-tiles
MI = 2 * H // 128        # 16 init m-tiles

# bias tile column layout ([128, NBIAS] fp32)
B_INIT = 0      # 16: fc_init_b
B_RZ1 = 16      # 16: bhh1[:2H]
B_N1H = 32      # 8:  bhh1[2H:]
B_IH1 = 40      # 24: bih1
B_RZ2 = 64      # 16: bih2[:2H]+bhh2[:2H]
B_N2H = 80      # 8:  bhh2[2H:]
B_N2I = 88      # 8:  bih2[2H:]
B_FCO = 96      # 2:  fco_b
NBIAS = 98

_cache = {}


def _wtiles(w_t, nm, nk):
    """[Kfull, Mfull] (already transposed: w_t = W.T) -> [nm, 128, nk*128]
    stationary-tile chunks: chunk[m][p, k*128+c] = w_t[128k+p, 128m+c]."""
    Kf, Mf = w_t.shape
    assert Kf == nk * 128 and Mf == nm * 128
    return np.ascontiguousarray(
        w_t.reshape(nk, 128, nm, 128).transpose(2, 1, 0, 3).reshape(nm, 128, nk * 128)
    ).astype(np.float16)


def _bias_cols(vec, n):
    """[n*128] -> [128, n] (col j = vec[128j:128j+128])"""
    return np.ascontiguousarray(vec.reshape(n, 128).T).astype(np.float32)


def build_program():
    import concourse.tile as tile
    from concourse import bacc, mybir

    f32, f16 = mybir.dt.float32, mybir.dt.float16
    Sig = mybir.ActivationFunctionType.Sigmoid
    Tanh = mybir.ActivationFunctionType.Tanh
    Ident = mybir.ActivationFunctionType.Identity

    nc = bacc.Bacc("TRN2", target_bir_lowering=False, debug=False,
                   enable_asserts=False, num_devices=NCORES)

    def din(name, shape, dt=f16):
        return nc.dram_tensor(name, shape, dt, kind="ExternalInput").ap()

    cflatT = din("cflatT", [KC, 128, R])
    prevT0 = din("prevT0", [S, 128, R])
    prevT1 = din("prevT1", [S, 128, R])
    w1h = din("w1h", [MG, 128, KH * 128])
    w2i = din("w2i", [MG, 128, KH * 128])
    w2h = din("w2h", [MG, 128, KH * 128])
    wp0 = din("wp0", [128, MG * 128])
    wp1 = din("wp1", [128, MG * 128])
    wc = din("wc", [MG, 128, KC * 128])
    wini = din("wini", [MI, 128, KC * 128])
    wfco = din("wfco", [128, KH * 256])
    biases = din("biases", [128, NBIAS], f32)
    yT = nc.dram_tensor("yT", [S, 132, R], f32, kind="ExternalOutput").ap()

    with tile.TileContext(nc) as tc:
        with tc.tile_pool(name="const", bufs=1) as const, \
             tc.tile_pool(name="stream", bufs=11) as stream, \
             tc.tile_pool(name="state", bufs=2) as state, \
             tc.tile_pool(name="gates", bufs=1) as gates, \
             tc.tile_pool(name="tmp", bufs=2) as tmp, \
             tc.tile_pool(name="ghb2p", bufs=9) as ghb2p, \
             tc.tile_pool(name="prevp", bufs=2) as prevp, \
             tc.tile_pool(name="outp", bufs=2) as outp, \
             tc.tile_pool(name="psA", bufs=5, space="PSUM") as psA, \
             tc.tile_pool(name="psD", bufs=3, space="PSUM") as psD:

            # ---- constant loads ----
            bias_sb = const.tile([128, NBIAS], f32, tag="bias")
            nc.sync.dma_start(bias_sb[:], biases[:])
            cfl_sb = const.tile([128, KC * R], f16, tag="cfl")
            for k in range(KC):
                nc.sync.dma_start(cfl_sb[:, k * R:(k + 1) * R], cflatT[k])
            wp0_sb = const.tile([128, MG * 128], f16, tag="wp0")
            nc.sync.dma_start(wp0_sb[:], wp0[:])
            wp1_sb = const.tile([128, MG * 128], f16, tag="wp1")
            nc.sync.dma_start(wp1_sb[:], wp1[:])
            wfco_sb = const.tile([128, KH * 256], f16, tag="wfco")
            nc.sync.dma_start(wfco_sb[:], wfco[:])

            def bias_ap(col):
                return bias_sb[:, col:col + 1]

            # ---- resident big weights (issues interleaved with init loads,
            # split across both HWDGE issuers so transfers overlap init compute)
            w1h_sb = const.tile([128, MG * KH * 128], f16, tag="w1h")
            w2i_sb = const.tile([128, MG * KH * 128], f16, tag="w2i")

            def weight_feed():
                for m in range(MG):
                    yield lambda m=m: nc.scalar.dma_start(
                        w1h_sb[:, m * 1024:(m + 1) * 1024], w1h[m])
                    yield lambda m=m: nc.scalar.dma_start(
                        w2i_sb[:, m * 1024:(m + 1) * 1024], w2i[m])
            wfeed = weight_feed()

            def feed(n=1):
                for _ in range(n):
                    f = next(wfeed, None)
                    if f is not None:
                        f()

            # ---- h init: t0T = tanh(wini @ cflatT + binit) ----
            h1T = state.tile([128, KH * R], f16, tag="h1")
            h2T = state.tile([128, KH * R], f16, tag="h2")
            for m in range(MI):
                wchunk = stream.tile([128, KC * 128], f16, tag="stream")
                nc.sync.dma_start(wchunk[:], wini[m])
                feed(1)
                ps = psA.tile([128, R], f32, tag="rz")
                for k in range(KC):
                    nc.tensor.matmul(ps[:], wchunk[:, k * 128:(k + 1) * 128],
                                     cfl_sb[:, k * R:(k + 1) * R],
                                     start=(k == 0), stop=(k == KC - 1))
                dst = h1T if m < NJ else h2T
                j = m % NJ
                nc.scalar.activation(dst[:, j * R:(j + 1) * R], ps[:], Tanh,
                                     bias=bias_ap(B_INIT + m))

            # ---- cached = Wc @ cflatT + bih1 ----
            cached_sb = const.tile([128, MG * R], f16, tag="cached")
            for m in range(MG):
                wchunk = stream.tile([128, KC * 128], f16, tag="stream")
                nc.sync.dma_start(wchunk[:], wc[m])
                feed(2)
                ps = psA.tile([128, R], f32, tag="rz")
                for k in range(KC):
                    nc.tensor.matmul(ps[:], wchunk[:, k * 128:(k + 1) * 128],
                                     cfl_sb[:, k * R:(k + 1) * R],
                                     start=(k == 0), stop=(k == KC - 1))
                nc.scalar.activation(cached_sb[:, m * R:(m + 1) * R], ps[:], Ident,
                                     bias=bias_ap(B_IH1 + m))
            feed(48)

            def w1h_t(m, k):
                return w1h_sb[:, m * 1024 + k * 128: m * 1024 + (k + 1) * 128]

            def w2i_t(m, k):
                return w2i_sb[:, m * 1024 + k * 128: m * 1024 + (k + 1) * 128]

            def hslice(t, j):
                return t[:, j * R:(j + 1) * R]

            def fco_step(h2T_cur, s):
                for mo, msz, osz, bc in [(0, 128, 128, B_FCO), (128, 32, 2, B_FCO + 1)]:
                    ps = psA.tile([128, R], f32, tag="rz")
                    for k in range(KH):
                        nc.tensor.matmul(ps[0:msz, :],
                                         wfco_sb[:, k * 256 + mo: k * 256 + mo + msz],
                                         hslice(h2T_cur, k),
                                         start=(k == 0), stop=(k == KH - 1))
                    ysb = outp.tile([128, R], f32, tag="y")
                    nc.scalar.activation(ysb[0:osz, :], ps[0:osz, :], Ident,
                                         bias=bias_sb[0:osz, bc:bc + 1])
                    nc.sync.dma_start(yT[s, mo:mo + osz, :], ysb[0:osz, :])

            h2T_done = []  # (h2T tile, step) pending fco

            for s in range(S):
                pv0 = prevp.tile([128, R], f16, tag="pv0")
                nc.sync.dma_start(pv0[:], prevT0[s])
                pv1 = prevp.tile([128, R], f16, tag="pv1")
                nc.sync.dma_start(pv1[:], prevT1[s])

                # ---------- GRU1 ----------
                r1 = gates.tile([128, NJ * R], f16, tag="rg")
                z1 = gates.tile([128, NJ * R], f16, tag="zg")
                n1 = gates.tile([128, NJ * R], f16, tag="ng")
                for m in range(MRZ):
                    ps = psA.tile([128, R], f32, tag="rz")
                    nc.tensor.matmul(ps[:], wp0_sb[:, m * 128:(m + 1) * 128], pv0[:],
                                     start=True, stop=False)
                    nc.tensor.matmul(ps[:], wp1_sb[:, m * 128:(m + 1) * 128], pv1[:],
                                     start=False, stop=False)
                    for k in range(KH):
                        nc.tensor.matmul(ps[:], w1h_t(m, k), hslice(h1T, k),
                                         start=False, stop=(k == KH - 1))
                    nc.vector.tensor_add(ps[:], ps[:], cached_sb[:, m * R:(m + 1) * R])
                    dst = r1 if m < NJ else z1
                    nc.scalar.activation(hslice(dst, m % NJ), ps[:], Sig,
                                         bias=bias_ap(B_RZ1 + m))
                # n-gate: all gh groups first, then the (tiny) gi groups
                h1T_new = state.tile([128, KH * R], f16, tag="h1")
                for j in range(NJ):
                    m = MRZ + j
                    psh = psD.tile([128, R], f32, tag="ghn")
                    for k in range(KH):
                        nc.tensor.matmul(psh[:], w1h_t(m, k), hslice(h1T, k),
                                         start=(k == 0), stop=(k == KH - 1))
                    ghb = ghb2p.tile([128, R], f16, tag="ghb")
                    nc.scalar.activation(ghb[:], psh[:], Ident, bias=bias_ap(B_N1H + j))
                    psg = psA.tile([128, R], f32, tag="rz")
                    nc.tensor.matmul(psg[:], wp0_sb[:, m * 128:(m + 1) * 128], pv0[:],
                                     start=True, stop=False)
                    nc.tensor.matmul(psg[:], wp1_sb[:, m * 128:(m + 1) * 128], pv1[:],
                                     start=False, stop=True)
                    nc.vector.tensor_add(psg[:], psg[:], cached_sb[:, m * R:(m + 1) * R])
                    tt = tmp.tile([128, R], f16, tag="tt")
                    nc.vector.tensor_mul(tt[:], hslice(r1, j), ghb[:])
                    nc.vector.tensor_add(psg[:], psg[:], tt[:])
                    nc.scalar.activation(hslice(n1, j), psg[:], Tanh, bias=0.0)
                    d = tmp.tile([128, R], f16, tag="d")
                    nc.vector.tensor_sub(d[:], hslice(h1T, j), hslice(n1, j))
                    nc.vector.tensor_mul(d[:], hslice(z1, j), d[:])
                    nc.vector.tensor_add(hslice(h1T_new, j), hslice(n1, j), d[:])

                # fco for previous step (PE filler while GRU1 gates finish)
                if h2T_done:
                    fco_step(*h2T_done.pop())

                # ---------- GRU2 ----------
                r2 = gates.tile([128, NJ * R], f16, tag="rg")
                z2 = gates.tile([128, NJ * R], f16, tag="zg")
                n2 = gates.tile([128, NJ * R], f16, tag="ng")
                # n-gate gh part first: independent of h1T_new
                ghb2s = []
                for j in range(NJ):
                    m = MRZ + j
                    wch = stream.tile([128, KH * 128], f16, tag="stream")
                    (nc.sync if j % 2 == 0 else nc.scalar).dma_start(wch[:], w2h[m])
                    psh = psD.tile([128, R], f32, tag="ghn")
                    for k in range(KH):
                        nc.tensor.matmul(psh[:], wch[:, k * 128:(k + 1) * 128],
                                         hslice(h2T, k),
                                         start=(k == 0), stop=(k == KH - 1))
                    ghb2 = ghb2p.tile([128, R], f16, tag="ghb2")
                    nc.scalar.activation(ghb2[:], psh[:], Ident, bias=bias_ap(B_N2H + j))
                    ghb2s.append(ghb2)
                for m in range(MRZ):
                    wch = stream.tile([128, KH * 128], f16, tag="stream")
                    (nc.sync if m % 2 == 0 else nc.scalar).dma_start(wch[:], w2h[m])
                    ps = psA.tile([128, R], f32, tag="rz")
                    for k in range(KH):
                        nc.tensor.matmul(ps[:], wch[:, k * 128:(k + 1) * 128],
                                         hslice(h2T, k),
                                         start=(k == 0), stop=False)
                    for k in range(KH):
                        nc.tensor.matmul(ps[:], w2i_t(m, k), hslice(h1T_new, k),
                                         start=False, stop=(k == KH - 1))
                    dst = r2 if m < NJ else z2
                    nc.scalar.activation(hslice(dst, m % NJ), ps[:], Sig,
                                         bias=bias_ap(B_RZ2 + m))
                h2T_new = state.tile([128, KH * R], f16, tag="h2")
                for j in range(NJ):
                    m = MRZ + j
                    psg = psD.tile([128, R], f32, tag="ghn")
                    for k in range(KH):
                        nc.tensor.matmul(psg[:], w2i_t(m, k), hslice(h1T_new, k),
                                         start=(k == 0), stop=(k == KH - 1))
                    tt = tmp.tile([128, R], f16, tag="tt")
                    nc.vector.tensor_mul(tt[:], hslice(r2, j), ghb2s[j][:])
                    nc.vector.tensor_add(psg[:], psg[:], tt[:])
                    nc.scalar.activation(hslice(n2, j), psg[:], Tanh,
                                         bias=bias_ap(B_N2I + j))
                    d = tmp.tile([128, R], f16, tag="d")
                    nc.vector.tensor_sub(d[:], hslice(h2T, j), hslice(n2, j))
                    nc.vector.tensor_mul(d[:], hslice(z2, j), d[:])
                    nc.vector.tensor_add(hslice(h2T_new, j), hslice(n2, j), d[:])

                h1T, h2T = h1T_new, h2T_new
                h2T_done.append((h2T, s))

            fco_step(*h2T_done.pop())

    nc.compile()
    return nc


def prep_inputs(c, target, fc_init_w, fc_init_b, g1_wih, g1_whh, g1_bih, g1_bhh,
                g2_wih, g2_whh, g2_bih, g2_bhh, fco_w, fco_b):
    """Host-side shard/layout prep. Returns per-core input maps."""
    f16 = np.float16
    T = E * S
    c = np.asarray(c, np.float32)
    target = np.asarray(target, np.float32)

    # weights: stationary-tile layouts (shared across cores)
    w1h_a = _wtiles(np.asarray(g1_whh, np.float32).T, MG, KH)
    w2i_a = _wtiles(np.asarray(g2_wih, np.float32).T, MG, KH)
    w2h_a = _wtiles(np.asarray(g2_whh, np.float32).T, MG, KH)
    wc_a = _wtiles(np.asarray(g1_wih, np.float32)[:, :C].T, MG, KC)
    wini_a = _wtiles(np.asarray(fc_init_w, np.float32).T, MI, KC)
    wp_t = np.asarray(g1_wih, np.float32)[:, C:].T      # [130, 3072]
    wp0_a = np.ascontiguousarray(wp_t[:128]).astype(f16)   # [128, 24*128]
    wp1_a = np.zeros((128, MG * 128), np.float32)
    wp1_a[:2] = wp_t[128:]
    wp1_a = wp1_a.astype(f16)
    wfco_a = np.zeros((128, KH, 256), np.float32)
    wfco_a[:, :, :D] = np.asarray(fco_w, np.float32).T.reshape(
        KH, 128, D).transpose(1, 0, 2)
    wfco_a = np.ascontiguousarray(wfco_a.reshape(128, KH * 256)).astype(f16)

    bias = np.zeros((128, NBIAS), np.float32)
    bias[:, B_INIT:B_INIT + MI] = _bias_cols(np.asarray(fc_init_b, np.float32), MI)
    bhh1 = np.asarray(g1_bhh, np.float32)
    bih1 = np.asarray(g1_bih, np.float32)
    bhh2 = np.asarray(g2_bhh, np.float32)
    bih2 = np.asarray(g2_bih, np.float32)
    bias[:, B_RZ1:B_RZ1 + 16] = _bias_cols(bhh1[:2 * H], 16)
    bias[:, B_N1H:B_N1H + 8] = _bias_cols(bhh1[2 * H:], 8)
    bias[:, B_IH1:B_IH1 + 24] = _bias_cols(bih1, 24)
    bias[:, B_RZ2:B_RZ2 + 16] = _bias_cols(bih2[:2 * H] + bhh2[:2 * H], 16)
    bias[:, B_N2H:B_N2H + 8] = _bias_cols(bhh2[2 * H:], 8)
    bias[:, B_N2I:B_N2I + 8] = _bias_cols(bih2[2 * H:], 8)
    fco_b = np.asarray(fco_b, np.float32)
    bias[:, B_FCO] = fco_b[:128]
    bias[0:2, B_FCO + 1] = fco_b[128:130]

    prev_full = np.concatenate(
        [np.zeros((B, 1, D), np.float32), target[:, :T - 1]], axis=1)  # [B,T,D]

    in_maps = []
    for core in range(NCORES):
        e0 = core * EPC
        cf = c[e0:e0 + EPC].reshape(R, C)                  # [256, 512]
        cfT = np.ascontiguousarray(cf.T.reshape(KC, 128, R)).astype(f16)
        pv = prev_full[:, e0 * S:(e0 + EPC) * S]           # [B, 32, D]
        pv = pv.reshape(B, EPC, S, D).transpose(2, 1, 0, 3).reshape(S, R, D)
        pvT = np.ascontiguousarray(pv.transpose(0, 2, 1))  # [S, D, R]
        pvT1_pad = np.zeros((S, 128, R), np.float32)
        pvT1_pad[:, :2] = pvT[:, 128:130]
        in_maps.append({
            "cflatT": cfT,
            "prevT0": np.ascontiguousarray(pvT[:, :128]).astype(f16),
            "prevT1": pvT1_pad.astype(f16),
            "w1h": w1h_a, "w2i": w2i_a, "w2h": w2h_a,
            "wp0": wp0_a, "wp1": wp1_a, "wc": wc_a, "wini": wini_a,
            "wfco": wfco_a, "biases": bias,
        })
    return in_maps


def assemble_output(results):
    """Per-core yT [S, 132, R] f32 -> full [B, T, D] f32."""
    T = E * S
    out = np.empty((B, T, D), np.float32)
    for core in range(NCORES):
        yt = results[core]["yT"]            # [S, 132, R]
        for ei in range(EPC):
            e = core * EPC + ei
            blk = yt[:, :D, ei * 128:(ei + 1) * 128]   # [S, D, 128]
            out[:, e * S:(e + 1) * S, :] = blk.transpose(2, 0, 1)
    return out


def kernel(c, target, length, batch_size, fc_init_w, fc_init_b,
           g1_wih, g1_whh, g1_bih, g1_bhh,
           g2_wih, g2_whh, g2_bih, g2_bhh, fco_w, fco_b):
    from concourse.bass_utils import run_bass_kernel_spmd

    if "nc" not in _cache:
        _cache["nc"] = build_program()
    nc = _cache["nc"]
    in_maps = prep_inputs(c, target, fc_init_w, fc_init_b,
                          g1_wih, g1_whh, g1_bih, g1_bhh,
                          g2_wih, g2_whh, g2_bih, g2_bhh, fco_w, fco_b)
    res = run_bass_kernel_spmd(nc, in_maps, list(range(NCORES)))
    return assemble_output(res.results)



# revision 3
# speedup vs baseline: 1.2876x; 1.2876x over previous
"""Trainium2 Bass kernel for nn_BottomLevelDecoderRNN.

2-layer GRU decoder, H=1024, S=16 steps, E*B = 2048 independent sequences,
data-parallel over 8 NeuronCores (R = 256 rows per core), everything kept
transposed as [feature, row].

Per-step math (per core):
  GRU1 r/z: ps = wp0@pv0 + wp1@pv1 + DR(w1h_rz, h1_8)   [fp8 DoubleRow, x256]
            + cached_rz (DVE)  -> sigmoid(ps/256 + bhh1_rz)
  GRU1 n:   psh = w1h_n@h1 (fp16) -> ghb;  psg = wp_n@pv + cached_n + r*ghb
            -> tanh -> h1' = n + z*(h1-n)  (batched DVE) -> fp8 copy (gpsimd)
  GRU2 r/z: ps = DR(w2h_rz, h2_8) + DR(w2i_rz, h1'_8) -> sigmoid(ps/256 + b)
  GRU2 n:   fp16 as GRU1.  fco output GEMM in fp16.

fp8 path: weights scaled x256 into e4m3 (clip 240), h states cast to e4m3
unscaled; PSUM carries 256x pre-activations, descaled by the activation's
scale=1/256. The n-gate GEMMs (error-sensitive) stay fp16. All big weights
are SBUF-resident (fp8 rz halves + fp16 n thirds fit in ~96KB/partition).
"""
import numpy as np

E, B, C, H, D = 16, 128, 512, 1024, 130
S = 16
NCORES = 8
EPC = E // NCORES        # 2 embeddings per core
R = EPC * B              # 256 rows per core
KH = H // 128            # 8 h k-tiles
KP = KH // 2             # 4 DR k-pairs
MG = 3 * H // 128        # 24 gate m-tiles
MRZ = 2 * H // 128       # 16 rz m-tiles
NJ = H // 128            # 8 n/h tiles
KC = C // 128            # 4 c k-tiles
MI = 2 * H // 128        # 16 init m-tiles
WS = 256.0               # fp8 weight scale

# bias tile column layout ([128, NBIAS] fp32)
B_INIT = 0      # 16: fc_init_b
B_RZ1 = 16      # 16: bhh1[:2H]
B_N1H = 32      # 8:  bhh1[2H:]
B_IH1 = 40      # 24: bih1 (rz entries pre-scaled x256)
B_RZ2 = 64      # 16: bih2[:2H]+bhh2[:2H]
B_N2H = 80      # 8:  bhh2[2H:]
B_N2I = 88      # 8:  bih2[2H:]
B_FCO = 96      # 2:  fco_b
NBIAS = 98

_cache = {}


def _wtiles(w_t, nm, nk):
    """[K, M] (w_t = W.T) -> [nm, 128, nk*128] fp16 stationary chunks."""
    Kf, Mf = w_t.shape
    assert Kf == nk * 128 and Mf == nm * 128
    return np.ascontiguousarray(
        w_t.reshape(nk, 128, nm, 128).transpose(2, 1, 0, 3).reshape(nm, 128, nk * 128)
    ).astype(np.float16)


def _wtiles_dr(w, scale=WS):
    """rz weight part [2H, H] -> [MRZ, 128, KP*2*128] fp8e4 DoubleRow chunks:
    chunk[m][p, kt, j, c] = (W.T)[kt*256 + j*128 + p, m*128 + c] * scale."""
    import ml_dtypes
    wt = np.asarray(w, np.float32).T * scale          # [H, 2H]
    arr = wt.reshape(KP, 2, 128, MRZ, 128).transpose(3, 2, 0, 1, 4)
    arr = np.clip(arr, -240, 240).astype(ml_dtypes.float8_e4m3)
    return np.ascontiguousarray(arr.reshape(MRZ, 128, KP * 2 * 128))


def _bias_cols(vec, n):
    return np.ascontiguousarray(vec.reshape(n, 128).T).astype(np.float32)


def build_program():
    import concourse.tile as tile
    from concourse import bacc, mybir

    f32, f16, f8 = mybir.dt.float32, mybir.dt.float16, mybir.dt.float8e4
    Sig = mybir.ActivationFunctionType.Sigmoid
    Tanh = mybir.ActivationFunctionType.Tanh
    Ident = mybir.ActivationFunctionType.Identity
    DRow = mybir.MatmulPerfMode.DoubleRow

    nc = bacc.Bacc("TRN2", target_bir_lowering=False, debug=False,
                   enable_asserts=False, num_devices=NCORES)

    def din(name, shape, dt=f16):
        return nc.dram_tensor(name, shape, dt, kind="ExternalInput").ap()

    cflatT = din("cflatT", [KC, 128, R])
    prevT0 = din("prevT0", [S, 128, R])
    prevT1 = din("prevT1", [S, 128, R])
    w1h8 = din("w1h8", [MRZ, 128, KP * 2 * 128], f8)
    w2i8 = din("w2i8", [MRZ, 128, KP * 2 * 128], f8)
    w2h8 = din("w2h8", [MRZ, 128, KP * 2 * 128], f8)
    w1hn = din("w1hn", [NJ, 128, KH * 128])
    w2in = din("w2in", [NJ, 128, KH * 128])
    w2hn = din("w2hn", [NJ, 128, KH * 128])
    wp0 = din("wp0", [128, MG * 128])
    wp1 = din("wp1", [128, MG * 128])
    wc = din("wc", [MG, 128, KC * 128])
    wini = din("wini", [MI, 128, KC * 128])
    wfco = din("wfco", [128, KH * 256])
    biases = din("biases", [128, NBIAS], f32)
    yT = nc.dram_tensor("yT", [S, 132, R], f32, kind="ExternalOutput").ap()

    with tile.TileContext(nc) as tc:
        with tc.tile_pool(name="const", bufs=1) as const, \
             tc.tile_pool(name="stream", bufs=4) as stream, \
             tc.tile_pool(name="state", bufs=2) as state, \
             tc.tile_pool(name="gates", bufs=2) as gates, \
             tc.tile_pool(name="tmp", bufs=2) as tmp, \
             tc.tile_pool(name="prevp", bufs=2) as prevp, \
             tc.tile_pool(name="outp", bufs=2) as outp, \
             tc.tile_pool(name="psA", bufs=5, space="PSUM") as psA, \
             tc.tile_pool(name="psD", bufs=3, space="PSUM") as psD:

            # ---- small constant loads ----
            bias_sb = const.tile([128, NBIAS], f32, tag="bias")
            nc.sync.dma_start(bias_sb[:], biases[:])
            cfl_sb = const.tile([128, KC * R], f16, tag="cfl")
            for k in range(KC):
                nc.sync.dma_start(cfl_sb[:, k * R:(k + 1) * R], cflatT[k])
            wp0_sb = const.tile([128, MG * 128], f16, tag="wp0")
            nc.sync.dma_start(wp0_sb[:], wp0[:])
            wp1_sb = const.tile([128, MG * 128], f16, tag="wp1")
            nc.sync.dma_start(wp1_sb[:], wp1[:])
            wfco_sb = const.tile([128, KH * 256], f16, tag="wfco")
            nc.sync.dma_start(wfco_sb[:], wfco[:])

            def bias_ap(col):
                return bias_sb[:, col:col + 1]

            # ---- resident big weights (fed on the scalar-issuer queue,
            # interleaved with init compute; ordered by first use) ----
            w1h8_sb = const.tile([128, MRZ, KP, 2, 128], f8, tag="w1h8")
            w2i8_sb = const.tile([128, MRZ, KP, 2, 128], f8, tag="w2i8")
            w2h8_sb = const.tile([128, MRZ, KP, 2, 128], f8, tag="w2h8")
            w1hn_sb = const.tile([128, NJ, KH * 128], f16, tag="w1hn")
            w2in_sb = const.tile([128, NJ, KH * 128], f16, tag="w2in")
            w2hn_sb = const.tile([128, NJ, KH * 128], f16, tag="w2hn")

            def weight_feed():
                for m in range(MRZ):
                    yield lambda m=m: nc.scalar.dma_start(w1h8_sb[:, m], w1h8[m])
                for j in range(NJ):
                    yield lambda j=j: nc.scalar.dma_start(w1hn_sb[:, j], w1hn[j])
                for j in range(NJ):
                    yield lambda j=j: nc.scalar.dma_start(w2hn_sb[:, j], w2hn[j])
                for m in range(MRZ):
                    yield lambda m=m: nc.scalar.dma_start(w2h8_sb[:, m], w2h8[m])
                for m in range(MRZ):
                    yield lambda m=m: nc.scalar.dma_start(w2i8_sb[:, m], w2i8[m])
                for j in range(NJ):
                    yield lambda j=j: nc.scalar.dma_start(w2in_sb[:, j], w2in[j])
            wfeed = weight_feed()

            def feed(n=1):
                for _ in range(n):
                    f = next(wfeed, None)
                    if f is not None:
                        f()

            # ---- h init: t0T = tanh(wini @ cflatT + binit) ----
            h1T = state.tile([128, NJ, R], f16, tag="h1")
            h2T = state.tile([128, NJ, R], f16, tag="h2")
            h18 = state.tile([128, NJ, R], f8, tag="h18")
            h28 = state.tile([128, NJ, R], f8, tag="h28")
            for m in range(MI):
                wchunk = stream.tile([128, KC * 128], f16, tag="stream")
                nc.sync.dma_start(wchunk[:], wini[m])
                feed(2)
                ps = psA.tile([128, R], f32, tag="rz")
                for k in range(KC):
                    nc.tensor.matmul(ps[:], wchunk[:, k * 128:(k + 1) * 128],
                                     cfl_sb[:, k * R:(k + 1) * R],
                                     start=(k == 0), stop=(k == KC - 1))
                dst = h1T if m < NJ else h2T
                j = m % NJ
                nc.scalar.activation(dst[:, j], ps[:], Tanh,
                                     bias=bias_ap(B_INIT + m))

            # ---- cached = Wc @ cflatT + bih1 (rz part scaled x256) ----
            cached_sb = const.tile([128, MG, R], f16, tag="cached")
            for m in range(MG):
                wchunk = stream.tile([128, KC * 128], f16, tag="stream")
                nc.sync.dma_start(wchunk[:], wc[m])
                feed(2)
                ps = psA.tile([128, R], f32, tag="rz")
                for k in range(KC):
                    nc.tensor.matmul(ps[:], wchunk[:, k * 128:(k + 1) * 128],
                                     cfl_sb[:, k * R:(k + 1) * R],
                                     start=(k == 0), stop=(k == KC - 1))
                sc = WS if m < MRZ else 1.0
                nc.scalar.activation(cached_sb[:, m], ps[:], Ident,
                                     bias=bias_ap(B_IH1 + m), scale=sc)
            feed(72)
            nc.gpsimd.tensor_copy(h18[:], h1T[:])
            nc.gpsimd.tensor_copy(h28[:], h2T[:])

            def fco_step(h2T_cur, s):
                for mo, msz, osz, bc in [(0, 128, 128, B_FCO), (128, 32, 2, B_FCO + 1)]:
                    ps = psA.tile([128, R], f32, tag="rz")
                    for k in range(KH):
                        nc.tensor.matmul(ps[0:msz, :],
                                         wfco_sb[:, k * 256 + mo: k * 256 + mo + msz],
                                         h2T_cur[:, k],
                                         start=(k == 0), stop=(k == KH - 1))
                    ysb = outp.tile([128, R], f32, tag="y")
                    nc.scalar.activation(ysb[0:osz, :], ps[0:osz, :], Ident,
                                         bias=bias_sb[0:osz, bc:bc + 1])
                    nc.sync.dma_start(yT[s, mo:mo + osz, :], ysb[0:osz, :])

            h2T_done = []  # (h2T tile, step) pending fco

            for s in range(S):
                pv0 = prevp.tile([128, R], f16, tag="pv0")
                nc.sync.dma_start(pv0[:], prevT0[s])
                pv1 = prevp.tile([128, R], f16, tag="pv1")
                nc.sync.dma_start(pv1[:], prevT1[s])

                # ---------- GRU1 r/z (fp8 DR h-part, x256 PSUM) ----------
                r1 = gates.tile([128, NJ, R], f16, tag="rg")
                z1 = gates.tile([128, NJ, R], f16, tag="zg")
                for m in range(MRZ):
                    ps = psA.tile([128, R], f32, tag="rz")
                    nc.tensor.matmul(ps[:], wp0_sb[:, m * 128:(m + 1) * 128], pv0[:],
                                     start=True, stop=False)
                    nc.tensor.matmul(ps[:], wp1_sb[:, m * 128:(m + 1) * 128], pv1[:],
                                     start=False, stop=False)
                    for kt in range(KP):
                        nc.tensor.matmul(ps[:], w1h8_sb[:, m, kt],
                                         h18[:, 2 * kt:2 * kt + 2, :],
                                         start=False, stop=(kt == KP - 1),
                                         perf_mode=DRow)
                    nc.vector.tensor_add(ps[:], ps[:], cached_sb[:, m])
                    dst = r1 if m < NJ else z1
                    nc.scalar.activation(dst[:, m % NJ], ps[:], Sig,
                                         bias=bias_ap(B_RZ1 + m), scale=1.0 / WS)

                # ---------- GRU1 n (fp16) ----------
                ghb1 = tmp.tile([128, NJ, R], f16, tag="ghb")
                for j in range(NJ):
                    psh = psD.tile([128, R], f32, tag="ghn")
                    for k in range(KH):
                        nc.tensor.matmul(psh[:], w1hn_sb[:, j, k * 128:(k + 1) * 128],
                                         h1T[:, k],
                                         start=(k == 0), stop=(k == KH - 1))
                    nc.scalar.activation(ghb1[:, j], psh[:], Ident,
                                         bias=bias_ap(B_N1H + j))
                # pre_n = cached_n + r1 * ghb1  (batched)
                tt1 = tmp.tile([128, NJ, R], f16, tag="tt")
                nc.vector.tensor_mul(tt1[:], r1[:], ghb1[:])
                nc.vector.tensor_add(tt1[:], tt1[:], cached_sb[:, MRZ:MG])
                n1 = gates.tile([128, NJ, R], f16, tag="ng")
                for j in range(NJ):
                    m = MRZ + j
                    psg = psA.tile([128, R], f32, tag="rz")
                    nc.tensor.matmul(psg[:], wp0_sb[:, m * 128:(m + 1) * 128], pv0[:],
                                     start=True, stop=False)
                    nc.tensor.matmul(psg[:], wp1_sb[:, m * 128:(m + 1) * 128], pv1[:],
                                     start=False, stop=True)
                    nc.vector.tensor_add(psg[:], psg[:], tt1[:, j])
                    nc.scalar.activation(n1[:, j], psg[:], Tanh, bias=0.0)
                # h1' = n1 + z1*(h1 - n1)  (batched)
                d1 = tmp.tile([128, NJ, R], f16, tag="tt")
                nc.vector.tensor_sub(d1[:], h1T[:], n1[:])
                nc.vector.tensor_mul(d1[:], z1[:], d1[:])
                h1T_new = state.tile([128, NJ, R], f16, tag="h1")
                nc.vector.tensor_add(h1T_new[:], n1[:], d1[:])
                h18_new = state.tile([128, NJ, R], f8, tag="h18")
                nc.gpsimd.tensor_copy(h18_new[:], h1T_new[:])

                # fco for previous step (PE filler)
                if h2T_done:
                    fco_step(*h2T_done.pop())

                # ---------- GRU2 n gh-part (fp16, uses old h2) ----------
                ghb2 = tmp.tile([128, NJ, R], f16, tag="ghb")
                for j in range(NJ):
                    psh = psD.tile([128, R], f32, tag="ghn")
                    for k in range(KH):
                        nc.tensor.matmul(psh[:], w2hn_sb[:, j, k * 128:(k + 1) * 128],
                                         h2T[:, k],
                                         start=(k == 0), stop=(k == KH - 1))
                    nc.scalar.activation(ghb2[:, j], psh[:], Ident,
                                         bias=bias_ap(B_N2H + j))

                # ---------- GRU2 r/z (all fp8 DR) ----------
                r2 = gates.tile([128, NJ, R], f16, tag="rg")
                z2 = gates.tile([128, NJ, R], f16, tag="zg")
                for m in range(MRZ):
                    ps = psA.tile([128, R], f32, tag="rz")
                    for kt in range(KP):
                        nc.tensor.matmul(ps[:], w2h8_sb[:, m, kt],
                                         h28[:, 2 * kt:2 * kt + 2, :],
                                         start=(kt == 0), stop=False,
                                         perf_mode=DRow)
                    for kt in range(KP):
                        nc.tensor.matmul(ps[:], w2i8_sb[:, m, kt],
                                         h18_new[:, 2 * kt:2 * kt + 2, :],
                                         start=False, stop=(kt == KP - 1),
                                         perf_mode=DRow)
                    dst = r2 if m < NJ else z2
                    nc.scalar.activation(dst[:, m % NJ], ps[:], Sig,
                                         bias=bias_ap(B_RZ2 + m), scale=1.0 / WS)

                # ---------- GRU2 n rest (fp16) ----------
                tt2 = tmp.tile([128, NJ, R], f16, tag="tt")
                nc.vector.tensor_mul(tt2[:], r2[:], ghb2[:])
                n2 = gates.tile([128, NJ, R], f16, tag="ng")
                for j in range(NJ):
                    psg = psD.tile([128, R], f32, tag="ghn")
                    for k in range(KH):
                        nc.tensor.matmul(psg[:], w2in_sb[:, j, k * 128:(k + 1) * 128],
                                         h1T_new[:, k],
                                         start=(k == 0), stop=(k == KH - 1))
                    nc.vector.tensor_add(psg[:], psg[:], tt2[:, j])
                    nc.scalar.activation(n2[:, j], psg[:], Tanh,
                                         bias=bias_ap(B_N2I + j))
                d2 = tmp.tile([128, NJ, R], f16, tag="tt")
                nc.vector.tensor_sub(d2[:], h2T[:], n2[:])
                nc.vector.tensor_mul(d2[:], z2[:], d2[:])
                h2T_new = state.tile([128, NJ, R], f16, tag="h2")
                nc.vector.tensor_add(h2T_new[:], n2[:], d2[:])
                h28_new = state.tile([128, NJ, R], f8, tag="h28")
                nc.gpsimd.tensor_copy(h28_new[:], h2T_new[:])

                h1T, h2T = h1T_new, h2T_new
                h18, h28 = h18_new, h28_new
                h2T_done.append((h2T, s))

            fco_step(*h2T_done.pop())

    nc.compile()
    return nc


def prep_inputs(c, target, fc_init_w, fc_init_b, g1_wih, g1_whh, g1_bih, g1_bhh,
                g2_wih, g2_whh, g2_bih, g2_bhh, fco_w, fco_b):
    """Host-side shard/layout prep. Returns per-core input maps."""
    f16 = np.float16
    T = E * S
    c = np.asarray(c, np.float32)
    target = np.asarray(target, np.float32)

    g1_whh = np.asarray(g1_whh, np.float32)
    g2_wih = np.asarray(g2_wih, np.float32)
    g2_whh = np.asarray(g2_whh, np.float32)
    w1h8_a = _wtiles_dr(g1_whh[:2 * H])
    w2i8_a = _wtiles_dr(g2_wih[:2 * H])
    w2h8_a = _wtiles_dr(g2_whh[:2 * H])
    w1hn_a = _wtiles(np.ascontiguousarray(g1_whh[2 * H:].T), NJ, KH)
    w2in_a = _wtiles(np.ascontiguousarray(g2_wih[2 * H:].T), NJ, KH)
    w2hn_a = _wtiles(np.ascontiguousarray(g2_whh[2 * H:].T), NJ, KH)
    wc_a = _wtiles(np.asarray(g1_wih, np.float32)[:, :C].T, MG, KC)
    wini_a = _wtiles(np.asarray(fc_init_w, np.float32).T, MI, KC)

    # prev-input weights: rz columns pre-scaled x256 to match the DR PSUM
    wp_t = np.asarray(g1_wih, np.float32)[:, C:].T.copy()   # [130, 3072]
    wp_t[:, :2 * H] *= WS
    wp0_a = np.ascontiguousarray(wp_t[:128]).astype(f16)
    wp1_a = np.zeros((128, MG * 128), np.float32)
    wp1_a[:2] = wp_t[128:]
    wp1_a = wp1_a.astype(f16)
    wfco_a = np.zeros((128, KH, 256), np.float32)
    wfco_a[:, :, :D] = np.asarray(fco_w, np.float32).T.reshape(
        KH, 128, D).transpose(1, 0, 2)
    wfco_a = np.ascontiguousarray(wfco_a.reshape(128, KH * 256)).astype(f16)

    bias = np.zeros((128, NBIAS), np.float32)
    bias[:, B_INIT:B_INIT + MI] = _bias_cols(np.asarray(fc_init_b, np.float32), MI)
    bhh1 = np.asarray(g1_bhh, np.float32)
    bih1 = np.asarray(g1_bih, np.float32).copy()
    bhh2 = np.asarray(g2_bhh, np.float32)
    bih2 = np.asarray(g2_bih, np.float32)
    bias[:, B_RZ1:B_RZ1 + 16] = _bias_cols(bhh1[:2 * H], 16)
    bias[:, B_N1H:B_N1H + 8] = _bias_cols(bhh1[2 * H:], 8)
    bias[:, B_IH1:B_IH1 + 24] = _bias_cols(bih1, 24)
    bias[:, B_IH1:B_IH1 + 16] *= WS
    bias[:, B_RZ2:B_RZ2 + 16] = _bias_cols(bih2[:2 * H] + bhh2[:2 * H], 16)
    bias[:, B_N2H:B_N2H + 8] = _bias_cols(bhh2[2 * H:], 8)
    bias[:, B_N2I:B_N2I + 8] = _bias_cols(bih2[2 * H:], 8)
    fco_b = np.asarray(fco_b, np.float32)
    bias[:, B_FCO] = fco_b[:128]
    bias[0:2, B_FCO + 1] = fco_b[128:130]

    prev_full = np.concatenate(
        [np.zeros((B, 1, D), np.float32), target[:, :T - 1]], axis=1)  # [B,T,D]

    in_maps = []
    for core in range(NCORES):
        e0 = core * EPC
        cf = c[e0:e0 + EPC].reshape(R, C)                  # [256, 512]
        cfT = np.ascontiguousarray(cf.T.reshape(KC, 128, R)).astype(f16)
        pv = prev_full[:, e0 * S:(e0 + EPC) * S]           # [B, 32, D]
        pv = pv.reshape(B, EPC, S, D).transpose(2, 1, 0, 3).reshape(S, R, D)
        pvT = np.ascontiguousarray(pv.transpose(0, 2, 1))  # [S, D, R]
        pvT1_pad = np.zeros((S, 128, R), np.float32)
        pvT1_pad[:, :2] = pvT[:, 128:130]
        in_maps.append({
            "cflatT": cfT,
            "prevT0": np.ascontiguousarray(pvT[:, :128]).astype(f16),
            "prevT1": pvT1_pad.astype(f16),
            "w1h8": w1h8_a, "w2i8": w2i8_a, "w2h8": w2h8_a,
            "w1hn": w1hn_a, "w2in": w2in_a, "w2hn": w2hn_a,
            "wp0": wp0_a, "wp1": wp1_a, "wc": wc_a, "wini": wini_a,
            "wfco": wfco_a, "biases": bias,
        })
    return in_maps


def assemble_output(results):
    """Per-core yT [S, 132, R] f32 -> full [B, T, D] f32."""
    T = E * S
    out = np.empty((B, T, D), np.float32)
    for core in range(NCORES):
        yt = results[core]["yT"]            # [S, 132, R]
        for ei in range(EPC):
            e = core * EPC + ei
            blk = yt[:, :D, ei * 128:(ei + 1) * 128]   # [S, D, 128]
            out[:, e * S:(e + 1) * S, :] = blk.transpose(2, 0, 1)
    return out


def kernel(c, target, length, batch_size, fc_init_w, fc_init_b,
           g1_wih, g1_whh, g1_bih, g1_bhh,
           g2_wih, g2_whh, g2_bih, g2_bhh, fco_w, fco_b):
    from concourse.bass_utils import run_bass_kernel_spmd

    if "nc" not in _cache:
        _cache["nc"] = build_program()
    nc = _cache["nc"]
    in_maps = prep_inputs(c, target, fc_init_w, fc_init_b,
                          g1_wih, g1_whh, g1_bih, g1_bhh,
                          g2_wih, g2_whh, g2_bih, g2_bhh, fco_w, fco_b)
    res = run_bass_kernel_spmd(nc, in_maps, list(range(NCORES)))
    return assemble_output(res.results)


# revision 4
# speedup vs baseline: 1.3959x; 1.0841x over previous
"""Trainium2 Bass kernel for nn_BottomLevelDecoderRNN.

2-layer GRU decoder, H=1024, S=16 steps, E*B = 2048 independent sequences,
data-parallel over 8 NeuronCores (R = 256 rows per core), everything kept
transposed as [feature, row].

Per-step math (per core):
  GRU1 r/z: ps = wp0@pv0 + wp1@pv1 + DR(w1h_rz, h1_8)   [fp8 DoubleRow, x256]
            + cached_rz (DVE)  -> sigmoid(ps/256 + bhh1_rz)
  GRU1 n:   psh = w1h_n@h1 (fp16) -> ghb;  psg = wp_n@pv + cached_n + r*ghb
            -> tanh -> h1' = n + z*(h1-n)  (batched DVE) -> fp8 copy (gpsimd)
  GRU2 r/z: ps = DR(w2h_rz, h2_8) + DR(w2i_rz, h1'_8) -> sigmoid(ps/256 + b)
  GRU2 n:   fp16 as GRU1.  fco output GEMM in fp16.

fp8 path: weights scaled x256 into e4m3 (clip 240), h states cast to e4m3
unscaled; PSUM carries 256x pre-activations, descaled by the activation's
scale=1/256. The n-gate GEMMs (error-sensitive) stay fp16. All big weights
are SBUF-resident (fp8 rz halves + fp16 n thirds fit in ~96KB/partition).
"""
import numpy as np

E, B, C, H, D = 16, 128, 512, 1024, 130
S = 16
NCORES = 8
EPC = E // NCORES        # 2 embeddings per core
R = EPC * B              # 256 rows per core
KH = H // 128            # 8 h k-tiles
KP = KH // 2             # 4 DR k-pairs
MG = 3 * H // 128        # 24 gate m-tiles
MRZ = 2 * H // 128       # 16 rz m-tiles
NJ = H // 128            # 8 n/h tiles
KC = C // 128            # 4 c k-tiles
MI = 2 * H // 128        # 16 init m-tiles
WS = 256.0               # fp8 weight scale

# bias tile column layout ([128, NBIAS] fp32)
B_INIT = 0      # 16: fc_init_b
B_RZ1 = 16      # 16: bhh1[:2H]
B_N1H = 32      # 8:  bhh1[2H:]
B_IH1 = 40      # 24: bih1 (rz entries pre-scaled x256)
B_RZ2 = 64      # 16: bih2[:2H]+bhh2[:2H]
B_N2H = 80      # 8:  bhh2[2H:]
B_N2I = 88      # 8:  bih2[2H:]
B_FCO = 96      # 2:  fco_b
NBIAS = 98

_cache = {}


def _wtiles(w_t, nm, nk):
    """[K, M] (w_t = W.T) -> [nm, 128, nk*128] fp16 stationary chunks."""
    Kf, Mf = w_t.shape
    assert Kf == nk * 128 and Mf == nm * 128
    return np.ascontiguousarray(
        w_t.reshape(nk, 128, nm, 128).transpose(2, 1, 0, 3).reshape(nm, 128, nk * 128)
    ).astype(np.float16)


def _wtiles_dr(w, scale=WS):
    """rz weight part [2H, H] -> [MRZ, 128, KP*2*128] fp8e4 DoubleRow chunks:
    chunk[m][p, kt, j, c] = (W.T)[kt*256 + j*128 + p, m*128 + c] * scale."""
    import ml_dtypes
    wt = np.asarray(w, np.float32).T * scale          # [H, 2H]
    arr = wt.reshape(KP, 2, 128, MRZ, 128).transpose(3, 2, 0, 1, 4)
    arr = np.clip(arr, -240, 240).astype(ml_dtypes.float8_e4m3)
    return np.ascontiguousarray(arr.reshape(MRZ, 128, KP * 2 * 128))


def _bias_cols(vec, n):
    return np.ascontiguousarray(vec.reshape(n, 128).T).astype(np.float32)


def build_program():
    import concourse.tile as tile
    from concourse import bacc, mybir

    f32, f16, f8 = mybir.dt.float32, mybir.dt.float16, mybir.dt.float8e4
    Sig = mybir.ActivationFunctionType.Sigmoid
    Tanh = mybir.ActivationFunctionType.Tanh
    Ident = mybir.ActivationFunctionType.Identity
    DRow = mybir.MatmulPerfMode.DoubleRow

    nc = bacc.Bacc("TRN2", target_bir_lowering=False, debug=False,
                   enable_asserts=False, num_devices=NCORES)

    def din(name, shape, dt=f16):
        return nc.dram_tensor(name, shape, dt, kind="ExternalInput").ap()

    cflatT = din("cflatT", [KC, 128, R])
    prevT0 = din("prevT0", [S, 128, R])
    prevT1 = din("prevT1", [S, 128, R])
    w1h8 = din("w1h8", [MRZ, 128, KP * 2 * 128], f8)
    w2i8 = din("w2i8", [MRZ, 128, KP * 2 * 128], f8)
    w2h8 = din("w2h8", [MRZ, 128, KP * 2 * 128], f8)
    w1hn = din("w1hn", [NJ, 128, KH * 128])
    w2in = din("w2in", [NJ, 128, KH * 128])
    w2hn = din("w2hn", [NJ, 128, KH * 128])
    wp0 = din("wp0", [128, MG * 128])
    wp1 = din("wp1", [128, MG * 128])
    wc = din("wc", [MG, 128, KC * 128])
    wini = din("wini", [MI, 128, KC * 128])
    wfco = din("wfco", [128, KH * 256])
    biases = din("biases", [128, NBIAS], f32)
    yT = nc.dram_tensor("yT", [S, 132, R], f32, kind="ExternalOutput").ap()

    with tile.TileContext(nc) as tc:
        with tc.tile_pool(name="const", bufs=1) as const, \
             tc.tile_pool(name="stream", bufs=4) as stream, \
             tc.tile_pool(name="state", bufs=2) as state, \
             tc.tile_pool(name="gates", bufs=2) as gates, \
             tc.tile_pool(name="tmp", bufs=2) as tmp, \
             tc.tile_pool(name="prevp", bufs=2) as prevp, \
             tc.tile_pool(name="outp", bufs=2) as outp, \
             tc.tile_pool(name="psA", bufs=5, space="PSUM") as psA, \
             tc.tile_pool(name="psD", bufs=3, space="PSUM") as psD:

            # ---- small constant loads ----
            bias_sb = const.tile([128, NBIAS], f32, tag="bias")
            nc.sync.dma_start(bias_sb[:], biases[:])
            cfl_sb = const.tile([128, KC * R], f16, tag="cfl")
            for k in range(KC):
                nc.sync.dma_start(cfl_sb[:, k * R:(k + 1) * R], cflatT[k])
            wp0_sb = const.tile([128, MG * 128], f16, tag="wp0")
            nc.sync.dma_start(wp0_sb[:], wp0[:])
            wp1_sb = const.tile([128, MG * 128], f16, tag="wp1")
            nc.sync.dma_start(wp1_sb[:], wp1[:])
            wfco_sb = const.tile([128, KH * 256], f16, tag="wfco")
            nc.sync.dma_start(wfco_sb[:], wfco[:])

            def bias_ap(col):
                return bias_sb[:, col:col + 1]

            # ---- resident big weights (fed on the scalar-issuer queue,
            # interleaved with init compute; ordered by first use) ----
            w1h8_sb = const.tile([128, MRZ, KP, 2, 128], f8, tag="w1h8")
            w2i8_sb = const.tile([128, MRZ, KP, 2, 128], f8, tag="w2i8")
            w2h8_sb = const.tile([128, MRZ, KP, 2, 128], f8, tag="w2h8")
            w1hn_sb = const.tile([128, NJ, KH * 128], f16, tag="w1hn")
            w2in_sb = const.tile([128, NJ, KH * 128], f16, tag="w2in")
            w2hn_sb = const.tile([128, NJ, KH * 128], f16, tag="w2hn")

            def weight_feed():
                for m in range(MRZ):
                    yield lambda m=m: nc.scalar.dma_start(w1h8_sb[:, m], w1h8[m])
                for j in range(NJ):
                    yield lambda j=j: nc.scalar.dma_start(w1hn_sb[:, j], w1hn[j])
                for j in range(NJ):
                    yield lambda j=j: nc.scalar.dma_start(w2hn_sb[:, j], w2hn[j])
                for m in range(MRZ):
                    yield lambda m=m: nc.scalar.dma_start(w2h8_sb[:, m], w2h8[m])
                for m in range(MRZ):
                    yield lambda m=m: nc.scalar.dma_start(w2i8_sb[:, m], w2i8[m])
                for j in range(NJ):
                    yield lambda j=j: nc.scalar.dma_start(w2in_sb[:, j], w2in[j])
            wfeed = weight_feed()

            def feed(n=1):
                for _ in range(n):
                    f = next(wfeed, None)
                    if f is not None:
                        f()

            # ---- h init: t0T = tanh(wini @ cflatT + binit) ----
            h1T = state.tile([128, NJ, R], f16, tag="h1")
            h2T = state.tile([128, NJ, R], f16, tag="h2")
            h18 = state.tile([128, NJ, R], f8, tag="h18")
            h28 = state.tile([128, NJ, R], f8, tag="h28")
            for m in range(MI):
                wchunk = stream.tile([128, KC * 128], f16, tag="stream")
                nc.sync.dma_start(wchunk[:], wini[m])
                feed(2)
                ps = psA.tile([128, R], f32, tag="rz")
                for k in range(KC):
                    nc.tensor.matmul(ps[:], wchunk[:, k * 128:(k + 1) * 128],
                                     cfl_sb[:, k * R:(k + 1) * R],
                                     start=(k == 0), stop=(k == KC - 1))
                dst = h1T if m < NJ else h2T
                j = m % NJ
                nc.scalar.activation(dst[:, j], ps[:], Tanh,
                                     bias=bias_ap(B_INIT + m))

            # ---- cached = Wc @ cflatT + bih1 (rz part scaled x256) ----
            cached_sb = const.tile([128, MG, R], f16, tag="cached")
            for m in range(MG):
                wchunk = stream.tile([128, KC * 128], f16, tag="stream")
                nc.sync.dma_start(wchunk[:], wc[m])
                feed(2)
                ps = psA.tile([128, R], f32, tag="rz")
                for k in range(KC):
                    nc.tensor.matmul(ps[:], wchunk[:, k * 128:(k + 1) * 128],
                                     cfl_sb[:, k * R:(k + 1) * R],
                                     start=(k == 0), stop=(k == KC - 1))
                sc = WS if m < MRZ else 1.0
                nc.scalar.activation(cached_sb[:, m], ps[:], Ident,
                                     bias=bias_ap(B_IH1 + m), scale=sc)
            feed(72)
            nc.gpsimd.tensor_copy(h18[:], h1T[:])
            nc.gpsimd.tensor_copy(h28[:], h2T[:])

            def fco_step(h2T_cur, s):
                for mo, msz, osz, bc in [(0, 128, 128, B_FCO), (128, 32, 2, B_FCO + 1)]:
                    ps = psA.tile([128, R], f32, tag="rz")
                    for k in range(KH):
                        nc.tensor.matmul(ps[0:msz, :],
                                         wfco_sb[:, k * 256 + mo: k * 256 + mo + msz],
                                         h2T_cur[:, k],
                                         start=(k == 0), stop=(k == KH - 1))
                    ysb = outp.tile([128, R], f32, tag="y")
                    nc.scalar.activation(ysb[0:osz, :], ps[0:osz, :], Ident,
                                         bias=bias_sb[0:osz, bc:bc + 1])
                    nc.sync.dma_start(yT[s, mo:mo + osz, :], ysb[0:osz, :])

            h2T_done = []  # (h2T tile, step) pending fco

            for s in range(S):
                pv0 = prevp.tile([128, R], f16, tag="pv0")
                nc.sync.dma_start(pv0[:], prevT0[s])
                pv1 = prevp.tile([128, R], f16, tag="pv1")
                nc.sync.dma_start(pv1[:], prevT1[s])

                # ---------- GRU1 r/z (fp8 DR h-part, x256 PSUM) ----------
                r1 = gates.tile([128, NJ, R], f16, tag="rg")
                z1 = gates.tile([128, NJ, R], f16, tag="zg")
                for m in range(MRZ):
                    ps = psA.tile([128, R], f32, tag="rz")
                    nc.tensor.matmul(ps[:], wp0_sb[:, m * 128:(m + 1) * 128], pv0[:],
                                     start=True, stop=False)
                    nc.tensor.matmul(ps[:], wp1_sb[:, m * 128:(m + 1) * 128], pv1[:],
                                     start=False, stop=False)
                    for kt in range(KP):
                        nc.tensor.matmul(ps[:], w1h8_sb[:, m, kt],
                                         h18[:, 2 * kt:2 * kt + 2, :],
                                         start=False, stop=(kt == KP - 1),
                                         perf_mode=DRow)
                    nc.vector.tensor_add(ps[:], ps[:], cached_sb[:, m])
                    dst = r1 if m < NJ else z1
                    nc.scalar.activation(dst[:, m % NJ], ps[:], Sig,
                                         bias=bias_ap(B_RZ1 + m), scale=1.0 / WS)

                # ---------- GRU1 n (fp16) ----------
                ghb1 = tmp.tile([128, NJ, R], f16, tag="ghb")
                for j in range(NJ):
                    psh = psD.tile([128, R], f32, tag="ghn")
                    for k in range(KH):
                        nc.tensor.matmul(psh[:], w1hn_sb[:, j, k * 128:(k + 1) * 128],
                                         h1T[:, k],
                                         start=(k == 0), stop=(k == KH - 1))
                    nc.scalar.activation(ghb1[:, j], psh[:], Ident,
                                         bias=bias_ap(B_N1H + j))
                # pre_n = cached_n + r1 * ghb1  (batched)
                tt1 = tmp.tile([128, NJ, R], f16, tag="tt")
                nc.vector.tensor_mul(tt1[:], r1[:], ghb1[:])
                nc.vector.tensor_add(tt1[:], tt1[:], cached_sb[:, MRZ:MG])
                n1 = gates.tile([128, NJ, R], f16, tag="ng")
                for j in range(NJ):
                    m = MRZ + j
                    psg = psA.tile([128, R], f32, tag="rz")
                    nc.tensor.matmul(psg[:], wp0_sb[:, m * 128:(m + 1) * 128], pv0[:],
                                     start=True, stop=False)
                    nc.tensor.matmul(psg[:], wp1_sb[:, m * 128:(m + 1) * 128], pv1[:],
                                     start=False, stop=True)
                    nc.vector.tensor_add(psg[:], psg[:], tt1[:, j])
                    nc.scalar.activation(n1[:, j], psg[:], Tanh, bias=0.0)
                # h1' = n1 + z1*(h1 - n1)  (batched)
                d1 = tmp.tile([128, NJ, R], f16, tag="tt")
                nc.vector.tensor_sub(d1[:], h1T[:], n1[:])
                nc.vector.tensor_mul(d1[:], z1[:], d1[:])
                h1T_new = state.tile([128, NJ, R], f16, tag="h1")
                nc.vector.tensor_add(h1T_new[:], n1[:], d1[:])
                h18_new = state.tile([128, NJ, R], f8, tag="h18")
                for j in range(NJ):
                    nc.vector.tensor_copy(h18_new[:, j], h1T_new[:, j])

                # fco for previous step (PE filler)
                if h2T_done:
                    fco_step(*h2T_done.pop())

                # ---------- GRU2 n gh-part (fp16, uses old h2) ----------
                ghb2 = tmp.tile([128, NJ, R], f16, tag="ghb")
                for j in range(NJ):
                    psh = psD.tile([128, R], f32, tag="ghn")
                    for k in range(KH):
                        nc.tensor.matmul(psh[:], w2hn_sb[:, j, k * 128:(k + 1) * 128],
                                         h2T[:, k],
                                         start=(k == 0), stop=(k == KH - 1))
                    nc.scalar.activation(ghb2[:, j], psh[:], Ident,
                                         bias=bias_ap(B_N2H + j))

                # ---------- GRU2 r/z (all fp8 DR) ----------
                r2 = gates.tile([128, NJ, R], f16, tag="rg")
                z2 = gates.tile([128, NJ, R], f16, tag="zg")
                for m in range(MRZ):
                    ps = psA.tile([128, R], f32, tag="rz")
                    for kt in range(KP):
                        nc.tensor.matmul(ps[:], w2h8_sb[:, m, kt],
                                         h28[:, 2 * kt:2 * kt + 2, :],
                                         start=(kt == 0), stop=False,
                                         perf_mode=DRow)
                    for kt in range(KP):
                        nc.tensor.matmul(ps[:], w2i8_sb[:, m, kt],
                                         h18_new[:, 2 * kt:2 * kt + 2, :],
                                         start=False, stop=(kt == KP - 1),
                                         perf_mode=DRow)
                    dst = r2 if m < NJ else z2
                    nc.scalar.activation(dst[:, m % NJ], ps[:], Sig,
                                         bias=bias_ap(B_RZ2 + m), scale=1.0 / WS)

                # ---------- GRU2 n rest (fp16) ----------
                tt2 = tmp.tile([128, NJ, R], f16, tag="tt")
                nc.vector.tensor_mul(tt2[:], r2[:], ghb2[:])
                n2 = gates.tile([128, NJ, R], f16, tag="ng")
                for j in range(NJ):
                    psg = psD.tile([128, R], f32, tag="ghn")
                    for k in range(KH):
                        nc.tensor.matmul(psg[:], w2in_sb[:, j, k * 128:(k + 1) * 128],
                                         h1T_new[:, k],
                                         start=(k == 0), stop=(k == KH - 1))
                    nc.vector.tensor_add(psg[:], psg[:], tt2[:, j])
                    nc.scalar.activation(n2[:, j], psg[:], Tanh,
                                         bias=bias_ap(B_N2I + j))
                d2 = tmp.tile([128, NJ, R], f16, tag="tt")
                nc.vector.tensor_sub(d2[:], h2T[:], n2[:])
                nc.vector.tensor_mul(d2[:], z2[:], d2[:])
                h2T_new = state.tile([128, NJ, R], f16, tag="h2")
                nc.vector.tensor_add(h2T_new[:], n2[:], d2[:])
                h28_new = state.tile([128, NJ, R], f8, tag="h28")
                nc.gpsimd.tensor_copy(h28_new[:], h2T_new[:])

                h1T, h2T = h1T_new, h2T_new
                h18, h28 = h18_new, h28_new
                h2T_done.append((h2T, s))

            fco_step(*h2T_done.pop())

    nc.compile()
    return nc


def prep_inputs(c, target, fc_init_w, fc_init_b, g1_wih, g1_whh, g1_bih, g1_bhh,
                g2_wih, g2_whh, g2_bih, g2_bhh, fco_w, fco_b):
    """Host-side shard/layout prep. Returns per-core input maps."""
    f16 = np.float16
    T = E * S
    c = np.asarray(c, np.float32)
    target = np.asarray(target, np.float32)

    g1_whh = np.asarray(g1_whh, np.float32)
    g2_wih = np.asarray(g2_wih, np.float32)
    g2_whh = np.asarray(g2_whh, np.float32)
    w1h8_a = _wtiles_dr(g1_whh[:2 * H])
    w2i8_a = _wtiles_dr(g2_wih[:2 * H])
    w2h8_a = _wtiles_dr(g2_whh[:2 * H])
    w1hn_a = _wtiles(np.ascontiguousarray(g1_whh[2 * H:].T), NJ, KH)
    w2in_a = _wtiles(np.ascontiguousarray(g2_wih[2 * H:].T), NJ, KH)
    w2hn_a = _wtiles(np.ascontiguousarray(g2_whh[2 * H:].T), NJ, KH)
    wc_a = _wtiles(np.asarray(g1_wih, np.float32)[:, :C].T, MG, KC)
    wini_a = _wtiles(np.asarray(fc_init_w, np.float32).T, MI, KC)

    # prev-input weights: rz columns pre-scaled x256 to match the DR PSUM
    wp_t = np.asarray(g1_wih, np.float32)[:, C:].T.copy()   # [130, 3072]
    wp_t[:, :2 * H] *= WS
    wp0_a = np.ascontiguousarray(wp_t[:128]).astype(f16)
    wp1_a = np.zeros((128, MG * 128), np.float32)
    wp1_a[:2] = wp_t[128:]
    wp1_a = wp1_a.astype(f16)
    wfco_a = np.zeros((128, KH, 256), np.float32)
    wfco_a[:, :, :D] = np.asarray(fco_w, np.float32).T.reshape(
        KH, 128, D).transpose(1, 0, 2)
    wfco_a = np.ascontiguousarray(wfco_a.reshape(128, KH * 256)).astype(f16)

    bias = np.zeros((128, NBIAS), np.float32)
    bias[:, B_INIT:B_INIT + MI] = _bias_cols(np.asarray(fc_init_b, np.float32), MI)
    bhh1 = np.asarray(g1_bhh, np.float32)
    bih1 = np.asarray(g1_bih, np.float32).copy()
    bhh2 = np.asarray(g2_bhh, np.float32)
    bih2 = np.asarray(g2_bih, np.float32)
    bias[:, B_RZ1:B_RZ1 + 16] = _bias_cols(bhh1[:2 * H], 16)
    bias[:, B_N1H:B_N1H + 8] = _bias_cols(bhh1[2 * H:], 8)
    bias[:, B_IH1:B_IH1 + 24] = _bias_cols(bih1, 24)
    bias[:, B_IH1:B_IH1 + 16] *= WS
    bias[:, B_RZ2:B_RZ2 + 16] = _bias_cols(bih2[:2 * H] + bhh2[:2 * H], 16)
    bias[:, B_N2H:B_N2H + 8] = _bias_cols(bhh2[2 * H:], 8)
    bias[:, B_N2I:B_N2I + 8] = _bias_cols(bih2[2 * H:], 8)
    fco_b = np.asarray(fco_b, np.float32)
    bias[:, B_FCO] = fco_b[:128]
    bias[0:2, B_FCO + 1] = fco_b[128:130]

    prev_full = np.concatenate(
        [np.zeros((B, 1, D), np.float32), target[:, :T - 1]], axis=1)  # [B,T,D]

    in_maps = []
    for core in range(NCORES):
        e0 = core * EPC
        cf = c[e0:e0 + EPC].reshape(R, C)                  # [256, 512]
        cfT = np.ascontiguousarray(cf.T.reshape(KC, 128, R)).astype(f16)
        pv = prev_full[:, e0 * S:(e0 + EPC) * S]           # [B, 32, D]
        pv = pv.reshape(B, EPC, S, D).transpose(2, 1, 0, 3).reshape(S, R, D)
        pvT = np.ascontiguousarray(pv.transpose(0, 2, 1))  # [S, D, R]
        pvT1_pad = np.zeros((S, 128, R), np.float32)
        pvT1_pad[:, :2] = pvT[:, 128:130]
        in_maps.append({
            "cflatT": cfT,
            "prevT0": np.ascontiguousarray(pvT[:, :128]).astype(f16),
            "prevT1": pvT1_pad.astype(f16),
            "w1h8": w1h8_a, "w2i8": w2i8_a, "w2h8": w2h8_a,
            "w1hn": w1hn_a, "w2in": w2in_a, "w2hn": w2hn_a,
            "wp0": wp0_a, "wp1": wp1_a, "wc": wc_a, "wini": wini_a,
            "wfco": wfco_a, "biases": bias,
        })
    return in_maps


def assemble_output(results):
    """Per-core yT [S, 132, R] f32 -> full [B, T, D] f32."""
    T = E * S
    out = np.empty((B, T, D), np.float32)
    for core in range(NCORES):
        yt = results[core]["yT"]            # [S, 132, R]
        for ei in range(EPC):
            e = core * EPC + ei
            blk = yt[:, :D, ei * 128:(ei + 1) * 128]   # [S, D, 128]
            out[:, e * S:(e + 1) * S, :] = blk.transpose(2, 0, 1)
    return out


def kernel(c, target, length, batch_size, fc_init_w, fc_init_b,
           g1_wih, g1_whh, g1_bih, g1_bhh,
           g2_wih, g2_whh, g2_bih, g2_bhh, fco_w, fco_b):
    from concourse.bass_utils import run_bass_kernel_spmd

    if "nc" not in _cache:
        _cache["nc"] = build_program()
    nc = _cache["nc"]
    in_maps = prep_inputs(c, target, fc_init_w, fc_init_b,
                          g1_wih, g1_whh, g1_bih, g1_bhh,
                          g2_wih, g2_whh, g2_bih, g2_bhh, fco_w, fco_b)
    res = run_bass_kernel_spmd(nc, in_maps, list(range(NCORES)))
    return assemble_output(res.results)
